# revision 1
# baseline (speedup 1.0000x reference)
"""Trainium2 Bass kernel for nn_AdaptiveFourierFeatures.

Strategy
--------
The reference module computes, per batch b and token s:

    q[s,h,:]   depends on x[s] through two linear layers
    k[d,f,h,:] = f[d,f]*u[h,:] + v[h,:]         (keys are AFFINE in f[d,f]
                                                  because key_proj is Linear(1,A))
    scores[s,d,h,f] = q.k/sqrt(HD) = alpha[s,h]*f[d,f] + beta[s,h]

With the given inputs, freq_matrix*freq_scale has IDENTICAL rows
(f[d,:] == g[:] for all d), so softmax over f is d-independent and beta
cancels inside the softmax:

    attn[s,h,f] = softmax_f(alpha[s,h] * (g[f]-gc))      (gc: shift for range)
    aw[s,f]     = mean_h attn[s,h,f]

The fourier features contract with the MLP weights analytically using
sin(theta+phi) = sin*cos + cos*sin, folding phase and the D dimension into
small [F,O] matrices on the host.  The device pipeline per token is then:

    x(64) -> alpha-scores(64=H*F) -> softmax -> aw features z(32)
          -> [x|z|1](97) @ G(97x128) -> sigmoid*silu gate -> residual

Device schedule (per core, one batch element, S=2048 tokens):
  - 4 column-chunks in a stacked-half layout (128 score rows = 2 halves x 64),
    emitted stage-major so all five engines pipeline across chunks.
  - engine assignment balances elementwise work: DVE does recip / z / wt,
    GpSimd (Pool) does attn-multiply / gate-multiply / residual, ACT does
    exp + tanh (exp_and_others table set only -- no table switch).
  - den and rb matmuls write back into the retiring s2 PSUM slot, keeping
    the whole softmax front in 3 PSUM banks.
  - residual x and the output travel as bf16 (tolerance is 2e-2; bf16
    rounding contributes ~4e-3).

Sharding: data-parallel over batch B=8, one batch element per NeuronCore.
All folded parameters are tiny and replicated.

kernel(**inputs) takes the FULL inputs and returns the FULL [B,S,D] output.
"""

import sys

import numpy as np
import ml_dtypes

# concourse (bass) lives in the trn repo; make sure it is importable even if
# the harness runs from a directory without the site defaults.
for _p in ("/opt/trn_rl_repo", "/opt/pypackages"):
    if _p not in sys.path:
        sys.path.append(_p)

# ---- problem constants (hardcoded; kernel.py must be self-contained) ----
B, S, D, F, A, H, O = 8, 2048, 64, 16, 32, 4, 64
HD = A // H
TWO_PI = 2.0 * np.pi
N_CORES = 8
HF = H * F            # 64 score columns per token
NFEAT = D + 2 * F + 1  # 97 = x | z_sin | z_cos | ones
HALF = S // 2          # stacked-half layout: 1024 tokens per half

BF16 = ml_dtypes.bfloat16
NCHUNKS = 2

_CACHE = {}


def _make_xT(xb: np.ndarray) -> np.ndarray:
    """[S, D] batch slice -> chunk-major transposed bf16 [D, S] layout."""
    xt = xb.T
    cw = HALF // NCHUNKS
    pieces = []
    for c in range(NCHUNKS):
        pieces.append(xt[:, c * cw:(c + 1) * cw])
        pieces.append(xt[:, HALF + c * cw:HALF + (c + 1) * cw])
    return np.ascontiguousarray(np.concatenate(pieces, axis=1)).astype(BF16)


def _make_inmaps(x: np.ndarray, params: dict) -> list:
    """Per-core input dicts for run_bass_kernel_spmd (shared w/ test.py)."""
    in_maps = []
    for b in range(N_CORES):
        m = dict(params)
        m["xT"] = _make_xT(x[b])
        in_maps.append(m)
    return in_maps


def _finish(x: np.ndarray, res) -> np.ndarray:
    """Host-side residual: out = x + 0.25 * gated4 (device returns gated*4)."""
    gs = np.stack([np.asarray(res.results[b]["out"]) for b in range(N_CORES)],
                  axis=0).astype(np.float32)
    return (x + 0.25 * gs).astype(np.float32)


def _build_program(nchunks: int = NCHUNKS, ndum: int = 2, dumn: int = 256):
    """Build the 8-core SPMD bass program (per-core shapes)."""
    import concourse.bass as bass
    import concourse.bacc as bacc
    import concourse.tile as tile
    from concourse import mybir

    dt = mybir.dt
    AF = mybir.ActivationFunctionType
    ALU = mybir.AluOpType

    nc = bacc.Bacc("TRN2", target_bir_lowering=False, debug=False,
                   enable_asserts=True, num_devices=N_CORES,
                   enable_partition_id=True)

    # ---- per-core DRAM parameters ----
    # all bf16 params packed into one [128, 361] array:
    #   wsc [64,64] @cols 0:64, o1 [128,8] @64:72, e2q [8,128] @72:200,
    #   o2 [128,32] @200:232, G [97,128] @232:360, b_score [128,1] @360
    xT = nc.dram_tensor("xT", [D, S], dt.bfloat16, kind="ExternalInput").ap()
    trig = nc.dram_tensor("trig", [2 * F, S], dt.bfloat16, kind="ExternalInput").ap()
    pk = nc.dram_tensor("pk", [128, 361], dt.bfloat16, kind="ExternalInput").ap()
    ones1 = nc.dram_tensor("ones1", [1, S], dt.bfloat16, kind="ExternalInput").ap()
    # device returns gated*4 in bf16; the residual add happens on the host
    out_d = nc.dram_tensor("out", [S, D], dt.bfloat16, kind="ExternalOutput").ap()

    KT = S // 128                # 16 token tiles of 128
    CW = HALF // nchunks         # stacked-column chunk width (512)
    kph = CW // 128              # k-tiles per half per chunk (4)
    KH = KT // 2                 # k-tiles per half (8)

    with tile.TileContext(nc) as tc:
        with (
            tc.tile_pool(name="const", bufs=1) as cpool,
            tc.tile_pool(name="sb", bufs=1) as sb,
            tc.tile_pool(name="work", bufs=3) as wk,
            tc.tile_pool(name="we1", bufs=4) as we1,
            tc.tile_pool(name="ps", bufs=2, space="PSUM") as ps,
            tc.tile_pool(name="ps_aw", bufs=2, space="PSUM") as psa,
            tc.tile_pool(name="ps_pre", bufs=2, space="PSUM") as psp,
        ):
            # ---- inputs to SBUF (x first -- it gates compute) ----
            # CZ = [x^T (0:64) | zs (64:80) | zc (80:96) | ones (96)]
            # xT arrives chunk-major from the host: chunk c occupies source
            # columns [c*2CW, (c+1)*2CW) = tokens {c*CW..} U {HALF+c*CW..},
            # so chunk 0's scores can start after the first piece lands.
            cz = sb.tile([NFEAT, S], dt.bfloat16)
            czx_v = cz[0:D, :].rearrange("d (h c) -> d h c", h=2)
            for c in range(nchunks):
                lo = c * CW
                nc.sync.dma_start(out=czx_v[:, :, lo:lo + CW],
                                  in_=xT[:, c * 2 * CW:(c + 1) * 2 * CW])

            # pk FIRST on the scalar(ACT) HWDGE ring, ahead of the implicit
            # ACT table load, so the score weights land with xT chunk 0.
            # pk arrives in two pieces: the score weights + exp bias (16KB)
            # first -- they gate the whole pipeline -- then the rest.
            c_pk = cpool.tile([128, 361], dt.bfloat16)
            nc.scalar.dma_start(out=c_pk[:, 0:65], in_=pk[:, 0:65])
            nc.scalar.dma_start(out=c_pk[:, 65:361], in_=pk[:, 65:361])
            c_wsc = c_pk[0:D, 0:64]
            c_o1 = c_pk[0:128, 65:73]
            c_e2q = c_pk[0:8, 73:201]
            c_o2 = c_pk[0:128, 201:233]
            c_g = c_pk[0:NFEAT, 233:361]

            # trig table lives on partitions 64..95 to lane-align with CZ
            c_trig = cpool.tile([96, S], dt.bfloat16)
            nc.sync.dma_start(out=c_trig[64:96, :], in_=trig[:])

            # ones row for the MLP bias arrives by DMA (a 1-partition Pool
            # memset costs ~2us of Q7 time)
            nc.scalar.dma_start(out=cz[NFEAT - 1:NFEAT, :], in_=ones1[:])

            # PE warm-up garbage tile (gates the dummy matmuls)
            wgarb = cpool.tile([128, 512], dt.bfloat16)
            nc.gpsimd.memset(wgarb[:], 0.0)

            # exp bias column (fp32 for the activation bias operand)
            c_bsc = cpool.tile([128, 1], dt.float32)
            nc.vector.tensor_copy(c_bsc[:], c_pk[:, 64:65])

            # f32r copy of the 1/den-broadcast matrix for the f32r matmul
            e2f = cpool.tile([8, 128], dt.float32r)
            nc.vector.tensor_copy(e2f[:], c_e2q)

            # warm up the activation table set (exp/tanh share one set)
            warm = cpool.tile([1, 2], dt.float32)
            nc.vector.memset(warm[:], 0.0)
            nc.scalar.activation(warm[:], warm[:], AF.Exp)

            # PE warm-up: a few full-width matmuls during the input-DMA wait
            # keep the HAM activity window busy toward the 2.4 GHz un-throttle
            pdum = ps.tile([128, CW], dt.float32, tag="fr")
            for _ in range(ndum):
                nc.tensor.matmul(pdum[:, 0:dumn], wgarb[:, 0:128],
                                 wgarb[:, 0:dumn], tile_position=(0, 0))

            from concourse.dve_ops import (
                RECIP_APPROX_FAST_CONSTS as _RC,
                RECIPROCAL_APPROX_FAST as _RAF,
            )

            # front stages are emitted STAGE-MAJOR across chunks so every
            # engine FIFO matches the data-ready order of the pipeline.
            ch = [dict(lo=c * CW, tok_los=(c * CW, HALF + c * CW))
                  for c in range(nchunks)]

            # -- scores: S2[half*64+hf, col] = sum_d x^T[d, tok] Wsc[d, hf]
            for st in ch:
                s2 = ps.tile([128, CW], dt.float32, tag="fr")
                st["s2"] = s2
                for h in range(2):
                    t0 = st["tok_los"][h]
                    nc.tensor.matmul(
                        s2[h * 64:(h + 1) * 64, :], c_wsc,
                        cz[0:D, t0:t0 + CW], tile_position=(0, h * 64),
                    )
            # -- exp (bias adds the constant alpha-offset term)
            for st in ch:
                e1 = we1.tile([128, CW], dt.bfloat16, tag="e1")
                st["e1"] = e1
                nc.scalar.activation(e1[:], st["s2"][:], AF.Exp, bias=c_bsc[:])
            # -- denominators, written into the retiring s2 slot (rows 0:8)
            for st in ch:
                den = st["s2"][0:8, :]
                st["den"] = den
                nc.tensor.matmul(den, c_o1, st["e1"][:], tile_position=(0, 0))
            # -- reciprocal (fast Newton approx, ~18 bits), f32r-rounded
            for st in ch:
                rec = we1.tile([8, CW], dt.float32r, tag="rec")
                st["rec"] = rec
                nc.vector._custom_dve(_RAF, out=rec[:], in0=st["den"],
                                      s0=_RC["s0"], s1=_RC["s1"],
                                      imm2=_RC["imm2"])
            # -- broadcast 1/den back to all 128 rows (x0.25 head-mean),
            #    overwriting the s2/den slot in place
            for st in ch:
                rb = st["s2"][:]
                st["rb"] = rb
                nc.tensor.matmul(rb, e2f[:], st["rec"][:],
                                 tile_position=(0, 0))
            # -- attn/4 = e1 * rb   (DVE: GPSIMD cannot read PSUM on TRN2)
            for st in ch:
                at = we1.tile([128, CW], dt.bfloat16, tag="at")
                st["at"] = at
                nc.vector.tensor_mul(at[:], st["e1"][:], st["rb"])
            # -- aw rows (duplicated for sin/cos) on partitions 64..95
            # each half's aw gets its own bank-aligned psum tile: a matmul
            # whose dest starts mid-bank faults the PE (found empirically)
            for st in ch:
                st["aw"] = []
                for h in range(2):
                    awh = psa.tile([96, CW], dt.float32, tag=f"awh{h}")
                    st["aw"].append(awh)
                    for n0 in range(0, CW, 512):
                        nn = min(512, CW - n0)
                        nc.tensor.matmul(
                            awh[64:96, n0:n0 + nn],
                            c_o2[h * 64:(h + 1) * 64, :],
                            st["at"][h * 64:(h + 1) * 64, n0:n0 + nn],
                            tile_position=(h * 64, 64),
                        )
            # -- z features into CZ rows 64..96, one op per half
            for st in ch:
                for h in range(2):
                    t0 = st["tok_los"][h]
                    nc.vector.tensor_mul(
                        cz[64:96, t0:t0 + CW],
                        st["aw"][h][64:96, :],
                        c_trig[64:96, t0:t0 + CW])

            # -- per-half tail: MLP -> tanh -> gates -> DMA out (gated*4)
            out_v = out_d.rearrange("(k p) d -> p k d", p=128)
            for c, st in enumerate(ch):
                for h in range(2):
                    k0 = st["tok_los"][h] // 128
                    pre = psp.tile([128, kph * 128], dt.float32, tag="pre")
                    for i in range(kph):
                        nc.tensor.matmul(
                            pre[:, i * 128:(i + 1) * 128],
                            cz[:, (k0 + i) * 128:(k0 + i + 1) * 128], c_g,
                            tile_position=(0, 0),
                        )
                    pre_v = pre[:].rearrange("p (j o) -> p j o", j=kph)
                    th = wk.tile([128, kph * 128], dt.bfloat16, tag="th")
                    th_v = th[:].rearrange("p (j o) -> p j o", j=kph)
                    # tanh(pre/2); sigmoid(a)=0.5+0.5*tanh(a/2)
                    nc.scalar.activation(th[:], pre[:], AF.Tanh, scale=0.5)
                    # w = (1+tanh_p) * pre_p   [silu*2]  (DVE stt)
                    wt = wk.tile([128, kph * 64], dt.bfloat16, tag="wt")
                    wt_v = wt[:].rearrange("p (j o) -> p j o", j=kph)
                    nc.vector.scalar_tensor_tensor(
                        wt_v, th_v[:, :, 64:128], 1.0, pre_v[:, :, 64:128],
                        ALU.add, ALU.mult,
                    )
                    # gated*4 = (1+tanh_g)*w = w + tanh_g*w on Pool
                    # (plain tensor_tensor only; all operands SBUF bf16)
                    gm = wk.tile([128, kph * 64], dt.bfloat16, tag="gm")
                    gm_v = gm[:].rearrange("p (j o) -> p j o", j=kph)
                    nc.gpsimd.tensor_mul(gm_v, th_v[:, :, 0:64], wt_v)
                    gs = wk.tile([128, kph * 64], dt.bfloat16, tag="gs")
                    nc.vector.tensor_add(gs[:], gm[:], wt[:])
                    # gated*4 out; residual is added on the host
                    eng = nc.sync if h == 0 else nc.scalar
                    eng.dma_start(out=out_v[:, k0:k0 + kph, :],
                                  in_=gs[:])

    nc.compile()
    return nc


def _fold_params(inputs):
    """Host-side constant folding (float64).  Returns per-core arrays."""
    f = (np.asarray(inputs["freq_matrix"], np.float64)
         * np.asarray(inputs["freq_scale"], np.float64))
    g = f[0]
    gc = 0.5 * (g.max() + g.min())
    gsh = g - gc

    Wq = np.asarray(inputs["Wq"], np.float64)
    bq = np.asarray(inputs["bq"], np.float64)
    Wk1 = np.asarray(inputs["Wk1"], np.float64)
    bk1 = np.asarray(inputs["bk1"], np.float64)
    Wqi = np.asarray(inputs["Wqi"], np.float64)
    bqi = np.asarray(inputs["bqi"], np.float64)
    Wki = np.asarray(inputs["Wki"], np.float64)
    bki = np.asarray(inputs["bki"], np.float64)
    ph = np.asarray(inputs["phase"], np.float64)

    u = Wki @ Wk1[:, 0]
    Wqq = Wqi @ Wq
    bqq = Wqi @ bq + bqi
    u_h = u.reshape(H, HD)
    M_alpha = np.einsum("he,hed->hd", u_h, Wqq.reshape(H, HD, D)) / np.sqrt(HD)
    c_alpha = np.einsum("he,he->h", u_h, bqq.reshape(H, HD)) / np.sqrt(HD)

    W_score = np.einsum("hd,f->dhf", M_alpha, gsh).reshape(D, HF)
    b_score = np.einsum("h,f->hf", c_alpha, gsh).reshape(HF)
    b_score2 = np.concatenate([b_score, b_score]).reshape(128, 1)

    t = np.linspace(0.0, 1.0, S)
    theta = TWO_PI * t[:, None] * g[None, :]
    trig = np.concatenate([np.sin(theta).T, np.cos(theta).T], 0)  # [2F, S]

    cph, sph = np.cos(ph), np.sin(ph)

    def fold_mlp(W):
        W = np.asarray(W, np.float64)
        Wx = W[:, :D]
        Wf = W[:, D:].reshape(O, D, 2 * F)
        Ws, Wc = Wf[:, :, :F], Wf[:, :, F:]
        Us = np.einsum("df,odf->fo", cph, Ws) - np.einsum("df,odf->fo", sph, Wc)
        Uc = np.einsum("df,odf->fo", sph, Ws) + np.einsum("df,odf->fo", cph, Wc)
        return Wx, Us, Uc

    Wgx, Ugs, Ugc = fold_mlp(inputs["Wg"])
    Wpx, Ups, Upc = fold_mlp(inputs["Wp"])
    bg = np.asarray(inputs["bg"], np.float64)
    bp = np.asarray(inputs["bp"], np.float64)

    G = np.zeros((NFEAT, 128))
    G[0:D, 0:64] = Wgx.T
    G[D:D + F, 0:64] = Ugs
    G[D + F:D + 2 * F, 0:64] = Ugc
    G[NFEAT - 1, 0:64] = bg
    G[0:D, 64:128] = Wpx.T
    G[D:D + F, 64:128] = Ups
    G[D + F:D + 2 * F, 64:128] = Upc
    G[NFEAT - 1, 64:128] = bp

    # indicator matrices for the softmax plumbing
    p = np.arange(128)
    O1 = (p[:, None] // 16 == np.arange(8)[None, :]).astype(np.float64)
    E2q = 0.25 * (np.arange(8)[:, None] == p[None, :] // 16).astype(np.float64)
    O2 = ((p[:, None] % 16) == (np.arange(32)[None, :] % 16)).astype(np.float64)

    # pack all bf16 params into one [128, 361] array (see _build_program):
    # [Wsc 0:64 | exp-bias 64 | O1 65:73 | E2q 73:201 | O2 201:233 | G 233:361]
    pk = np.zeros((128, 361))
    pk[0:D, 0:64] = W_score
    pk[:, 64] = b_score2[:, 0]
    pk[0:128, 65:73] = O1
    pk[0:8, 73:201] = E2q
    pk[0:128, 201:233] = O2
    pk[0:NFEAT, 233:361] = G

    return dict(
        trig=trig.astype(BF16),
        pk=pk.astype(BF16),
        ones1=np.ones((1, S), BF16),
    ), gsh, M_alpha, c_alpha


def _numpy_fallback(inputs):
    """Exact collapsed computation in numpy (general freq rows)."""
    x = np.asarray(inputs["x"], np.float64)
    f = (np.asarray(inputs["freq_matrix"], np.float64)
         * np.asarray(inputs["freq_scale"], np.float64))
    Wq = np.asarray(inputs["Wq"], np.float64); bq = np.asarray(inputs["bq"], np.float64)
    Wk1 = np.asarray(inputs["Wk1"], np.float64); bk1 = np.asarray(inputs["bk1"], np.float64)
    Wqi = np.asarray(inputs["Wqi"], np.float64); bqi = np.asarray(inputs["bqi"], np.float64)
    Wki = np.asarray(inputs["Wki"], np.float64); bki = np.asarray(inputs["bki"], np.float64)
    Wg = np.asarray(inputs["Wg"], np.float64); bg = np.asarray(inputs["bg"], np.float64)
    Wp = np.asarray(inputs["Wp"], np.float64); bp = np.asarray(inputs["bp"], np.float64)
    ph = np.asarray(inputs["phase"], np.float64)

    u = Wki @ Wk1[:, 0]
    v = Wki @ bk1 + bki
    q = (x @ Wq.T + bq) @ Wqi.T + bqi                      # [B,S,A]
    qh = q.reshape(B, S, H, HD)
    alpha = np.einsum("bshe,he->bsh", qh, u.reshape(H, HD)) / np.sqrt(HD)
    beta = np.einsum("bshe,he->bsh", qh, v.reshape(H, HD)) / np.sqrt(HD)
    sc = alpha[..., None, :, None] * f[None, None, :, None, :] \
        + beta[..., None, :, None]                         # [B,S,D,H,F]
    sc -= sc.max(-1, keepdims=True)
    e = np.exp(sc)
    attn = e / e.sum(-1, keepdims=True)
    aw = attn.mean(-2)                                     # [B,S,D,F]
    t = np.linspace(0.0, 1.0, S)
    sig = TWO_PI * t[None, :, None, None] * f[None, None] + ph[None, None]
    ffs = np.sin(sig) * aw
    ffc = np.cos(sig) * aw
    ff = np.concatenate([ffs, ffc], axis=-1).reshape(B, S, D * 2 * F)
    ci = np.concatenate([x, ff], axis=-1)
    gate = 1.0 / (1.0 + np.exp(-(ci @ Wg.T + bg)))
    pp = ci @ Wp.T + bp
    silu = pp / (1.0 + np.exp(-pp))
    return (x + gate * silu).astype(np.float32)


def kernel(**inputs) -> np.ndarray:
    x = np.asarray(inputs["x"], np.float32)

    f = (np.asarray(inputs["freq_matrix"], np.float64)
         * np.asarray(inputs["freq_scale"], np.float64))
    if not np.all(f == f[0:1]):
        return _numpy_fallback(inputs)

    params, gsh, M_alpha, c_alpha = _fold_params(inputs)

    # exp-overflow guard (score = alpha*(g-gc); needs |score| < ~85)
    xmaxn = np.linalg.norm(x.reshape(-1, D), axis=1).max()
    amax = np.linalg.norm(M_alpha, axis=1).max() * xmaxn + np.abs(c_alpha).max()
    if amax * np.abs(gsh).max() > 85.0:
        return _numpy_fallback(inputs)

    key = "prog"
    if key not in _CACHE:
        _CACHE[key] = _build_program()
    nc = _CACHE[key]

    from concourse.bass_utils import run_bass_kernel_spmd

    in_maps = _make_inmaps(x, params)
    res = run_bass_kernel_spmd(nc, in_maps, core_ids=list(range(N_CORES)))
    return _finish(x, res)


if __name__ == "__main__":
    import reference
    ins = {k: np.asarray(v) for k, v in reference.setup_inputs().items()}
    got = kernel(**ins)
    import jax.numpy as jnp
    exp = np.asarray(reference.reference(**{k: jnp.asarray(v) for k, v in ins.items()}))
    err = np.linalg.norm(got - exp) / np.linalg.norm(exp)
    print("rel err:", err)

